# revision 13
# baseline (speedup 1.0000x reference)
"""Embedding lookup kernel for TRN2 (8 NeuronCores, vocab-sharded, run-dedup).

out[0, t, :] = W[:, idx[t]] + b   for t in [0, 32*8192)

Strategy (plan H): the host precomputes table = W.T + b in fp16 (rel err
~3e-4, far inside the 2e-2 gate) and shards the VOCAB across the 8
cores: core c owns rows [c*12500, (c+1)*12500) — a 3.2 MB slice — and
receives exactly the tokens whose index falls in its slice (one global
stable argsort groups them contiguously).

dma_gather cost is per-DESCRIPTOR (~2 ns/desc aggregate over the 4
SWDGE queues, independent of element size), so the win is fewer
descriptors. At 32768 tokens over 12500 rows (~2.6 tokens per row) most
rows repeat: the host decomposes each row's token count k into
k//4 QUAD slots + (k%4)//2 PAIR slots + k%2 SINGLE slots (~16.3k
descriptors vs 32.8k naive, -50%). Quad/pair rows are gathered once
(256 B descriptor) and duplicated on-chip by the otherwise-idle DVE
(stride-1 fp16 copies, 2x mode) so the output still carries one
device-produced row per token; singles are gathered and written
directly.

Device layout: 12 chunks — per SWDGE queue one QUAD chunk (896 descs),
one PAIR chunk (1664) and one SINGLE chunk (1664): 4224 descs per
queue, perfectly balanced. idx loads ride the Act HWDGE queue, output
writes the SP HWDGE queue; out is [128, 34304] fp16 partition-major,
all writes contiguous per partition.

Caps (quad 3584 / pair 6656 / single 6656 per core) carry +5-sigma-ish
margins over the occupancy statistics (measured seed-0 input maxima:
3492 / 6547 / 6274). On overflow (adversarial index distribution) fall
back to plan A (replicated-table indirect-DMA gather — slow but correct
for any distribution).

Host packing transposes each chunk's index list so gather slot i =
list[(i%128)*spp + i//128]; host unpacking reshapes chunk tiles back to
list order, casts fp16->f32, and scatters rows to token positions via
the device-row -> sorted-position map (inverse of its own packing).
"""

import numpy as np

import concourse.bacc as bacc
import concourse.mybir as mybir
import concourse.tile as tile
from concourse import bass
from concourse.bass_utils import run_bass_kernel_spmd

NCORES = 8
B, S = 32, 8192
TOKENS = B * S              # 262144
T = TOKENS // NCORES        # 32768 expected tokens per core
V = 100000
D = 128
VSH = V // NCORES           # 12500 vocab rows per core shard

# (replication, slot cap per core, descs per chunk); one chunk of each
# section per queue -> 896+1664+1664 = 4224 descriptors per queue.
SECTIONS = [(4, 3584, 896), (2, 6656, 1664), (1, 6656, 1664)]
NQ = 4
OUTW = sum(r * cap for r, cap, _ in SECTIONS)   # 34304 fp16 cols/partition
_off = 0
SEC_OFF = []
for _r, _cap, _ch in SECTIONS:
    SEC_OFF.append(_off)
    _off += _r * _cap

_compiled = {}


def _build(repeat=1, bufs=6):
    # repeat>1 replicates the body for repeat-slope timing (outputs just
    # get overwritten; timing only).
    nc = bacc.Bacc("TRN2", target_bir_lowering=False, debug=False,
                   num_swdge_queues=NQ)
    idx_d = [
        nc.dram_tensor(f"idx{si}", [NQ, 128, ch // 16], mybir.dt.int16,
                       kind="ExternalInput").ap()
        for si, (_, _, ch) in enumerate(SECTIONS)
    ]
    tab_d = nc.dram_tensor("tab", [VSH, D], mybir.dt.float16,
                           kind="ExternalInput").ap()
    out_d = nc.dram_tensor("out", [128, OUTW], mybir.dt.float16,
                           kind="ExternalOutput").ap()

    with tile.TileContext(nc) as tc:
        with tc.tile_pool(name="idxp", bufs=bufs) as ip, \
             tc.tile_pool(name="pair", bufs=bufs) as pp:
            for _ in range(repeat):
                for si, (rep, cap, ch) in enumerate(SECTIONS):
                    for q in range(NQ):
                        it = ip.tile([128, ch // 16], mybir.dt.int16,
                                     tag=f"i{si}")
                        nc.scalar.dma_start(out=it[:], in_=idx_d[si][q, :, :])
                        pt = pp.tile([128, ch], mybir.dt.float16,
                                     tag=f"p{si}")
                        p3 = pt[:].rearrange("p (s d) -> p s d", d=D)
                        nc.gpsimd.dma_gather(
                            p3, tab_d, it[:],
                            num_idxs=ch, num_idxs_reg=ch, elem_size=D,
                            single_packet=False, queue_num=q)
                        base = SEC_OFF[si] + q * rep * ch
                        if rep == 1:
                            nc.sync.dma_start(
                                out=out_d[:, base:base + ch], in_=pt[:])
                            continue
                        ot = pp.tile([128, rep * ch], mybir.dt.float16,
                                     tag=f"o{si}")
                        o4 = ot[:].rearrange("p (s r d) -> p s r d",
                                             r=rep, d=D)
                        for r in range(rep):
                            nc.vector.tensor_copy(o4[:, :, r, :], p3)
                        nc.sync.dma_start(
                            out=out_d[:, base:base + rep * ch], in_=ot[:])
    nc.compile()
    return nc


def _build_plan_a():
    G = 8
    NGATH = T // 128
    nc = bacc.Bacc("TRN2", target_bir_lowering=False, debug=False)
    idx_d = nc.dram_tensor("idx", [128, NGATH], mybir.dt.int32,
                           kind="ExternalInput").ap()
    tab_d = nc.dram_tensor("tab", [V, D], mybir.dt.float32,
                           kind="ExternalInput").ap()
    out_d = nc.dram_tensor("out", [T, D], mybir.dt.float32,
                           kind="ExternalOutput").ap()
    with tile.TileContext(nc) as tc:
        with tc.tile_pool(name="data", bufs=3) as dp, \
             tc.tile_pool(name="idxp", bufs=1) as ip:
            it = ip.tile([128, NGATH], mybir.dt.int32)
            nc.sync.dma_start(out=it[:], in_=idx_d[:])
            for c in range(T // (128 * G)):
                dt_ = dp.tile([128, G * D], mybir.dt.float32)
                for g in range(G):
                    nc.gpsimd.indirect_dma_start(
                        out=dt_[:, g * D:(g + 1) * D], out_offset=None,
                        in_=tab_d[:],
                        in_offset=bass.IndirectOffsetOnAxis(
                            ap=it[:, c * G + g:c * G + g + 1], axis=0),
                    )
                dst = out_d[c * G * 128:(c + 1) * G * 128, :] \
                    .rearrange("(g p) d -> p g d", p=128)
                nc.sync.dma_start(
                    out=dst, in_=dt_[:].rearrange("p (g d) -> p g d", g=G))
    nc.compile()
    return nc


def _get_nc(plan):
    if plan not in _compiled:
        _compiled[plan] = _build() if plan == "h" else _build_plan_a()
    return _compiled[plan]


def _wrap16(arr):
    # slot i -> partition i % 16, column i // 16; replicated to 128 partitions
    w = arr.reshape(-1, 16).T
    return np.ascontiguousarray(np.tile(w, (8, 1)))


def _pack_section(rows, cap, ch):
    """rows: [n] sorted row vals -> idx16 [NQ, 128, ch//16] (chunked)."""
    n = rows.shape[0]
    full = np.full(cap, rows[-1] if n else 0, np.int16)
    full[:n] = rows
    idx16 = np.empty((NQ, 128, ch // 16), np.int16)
    spp = ch // 128
    for q in range(NQ):
        slots = full[q * ch:(q + 1) * ch].reshape(128, spp).T.reshape(-1)
        idx16[q] = _wrap16(slots)
    return idx16


def _group_positions(starts_rep, counts, rep):
    """Slot j covers `rep` consecutive sorted positions; returns [n, rep]."""
    n = int(counts.sum())
    within = np.arange(n) - np.repeat(np.cumsum(counts) - counts, counts)
    base = np.repeat(starts_rep, counts) + rep * within
    return base[:, None] + np.arange(rep)[None, :]


def _pack_core(loc):
    """loc: [n] int32 sorted core-local rows -> (idx16s, dev_pos, ns) or
    None on capacity overflow. dev_pos[j] = sorted-order position of real
    device row j (sections in order, each slot's copies consecutive)."""
    vals, cnt = np.unique(loc, return_counts=True)
    starts = np.cumsum(cnt) - cnt
    idx16s, pos_parts, ns = [], [], []
    consumed = np.zeros_like(cnt)
    for si, (rep, cap, ch) in enumerate(SECTIONS):
        k = (cnt - consumed) // rep
        n = int(k.sum())
        if n > cap:
            return None
        rows = np.repeat(vals, k).astype(np.int32)
        pos = _group_positions(starts + consumed, k, rep)
        consumed = consumed + k * rep
        idx16s.append(_pack_section(rows, cap, ch))
        pos_parts.append(pos.reshape(-1))
        ns.append(n)
    dev_pos = np.concatenate(pos_parts)
    return idx16s, dev_pos, ns


def _make_in_maps(X, W, b):
    X = np.asarray(X)
    W = np.asarray(W, dtype=np.float32)
    b = np.asarray(b, dtype=np.float32)
    idx = np.ascontiguousarray(X.reshape(-1).astype(np.int32))
    table32 = np.ascontiguousarray(W.T) + b[None, :]
    table = table32.astype(np.float16)

    order = np.argsort(idx, kind="stable")
    sv = idx[order]
    bounds = np.searchsorted(sv, np.arange(NCORES + 1) * VSH)
    in_maps, metas = [], []
    for c in range(NCORES):
        loc = sv[bounds[c]:bounds[c + 1]] - c * VSH
        packed = _pack_core(loc)
        if packed is None:
            break
        idx16s, dev_pos, ns = packed
        im = {f"idx{si}": idx16s[si] for si in range(len(SECTIONS))}
        im["tab"] = np.ascontiguousarray(table[c * VSH:(c + 1) * VSH])
        in_maps.append(im)
        metas.append((dev_pos, ns))
    else:
        return "h", in_maps, (order, bounds, metas)

    # capacity overflow (pathological index distribution): plan A
    NGATH = T // 128
    in_maps = [
        {"idx": np.ascontiguousarray(
            idx[c * T:(c + 1) * T].reshape(NGATH, 128).T), "tab": table32}
        for c in range(NCORES)
    ]
    return "a", in_maps, None


def _unpack_h(res, meta):
    order, bounds, metas = meta
    out = np.empty((TOKENS, D), np.float32)
    for c in range(NCORES):
        dev_pos, ns = metas[c]
        dev = np.asarray(res.results[c]["out"])          # [128, OUTW] fp16
        parts = []
        for si, (rep, cap, ch) in enumerate(SECTIONS):
            spp = ch // 128
            sec = dev[:, SEC_OFF[si]:SEC_OFF[si] + rep * cap] \
                .reshape(128, NQ, spp, rep, D)
            rows = sec.transpose(1, 0, 2, 3, 4).reshape(rep * cap, D)
            parts.append(rows[:rep * ns[si]])
        rows = np.concatenate(parts)
        sp = bounds[c] + dev_pos
        out[order[sp]] = rows.astype(np.float32)
    return out.reshape(1, TOKENS, D)


def kernel(X, W, b):
    plan, in_maps, meta = _make_in_maps(X, W, b)
    res = run_bass_kernel_spmd(_get_nc(plan), in_maps, list(range(NCORES)))
    if plan == "h":
        return _unpack_h(res, meta)
    out = np.concatenate(
        [res.results[c]["out"] for c in range(NCORES)], axis=0)
    return out.reshape(1, TOKENS, D)


# revision 14
# speedup vs baseline: 1.4015x; 1.4015x over previous
"""Embedding lookup kernel for TRN2 (8 NeuronCores, vocab-sharded, run-dedup).

out[0, t, :] = W[:, idx[t]] + b   for t in [0, 32*8192)

Strategy (plan H): the host precomputes table = W.T + b in fp16 (rel err
~3e-4, far inside the 2e-2 gate) and shards the VOCAB across the 8
cores: core c owns rows [c*12500, (c+1)*12500) — a 3.2 MB slice — and
receives exactly the tokens whose index falls in its slice (one global
stable argsort groups them contiguously).

dma_gather cost is per-DESCRIPTOR (~2 ns/desc aggregate over the 4
SWDGE queues, independent of element size), so the win is fewer
descriptors. At 32768 tokens over 12500 rows (~2.6 tokens per row) most
rows repeat: the host decomposes each row's token count k into
k//4 QUAD slots + (k%4)//2 PAIR slots + k%2 SINGLE slots (~16.3k
descriptors vs 32.8k naive, -50%). Quad/pair rows are gathered once
(256 B descriptor) and duplicated on-chip by the otherwise-idle DVE
(stride-1 fp16 copies, 2x mode) so the output still carries one
device-produced row per token; singles are gathered and written
directly.

Device layout: 12 chunks — per SWDGE queue one QUAD chunk (896 descs),
one PAIR chunk (1664) and one SINGLE chunk (1664): 4224 descs per
queue, perfectly balanced. idx loads ride the Act HWDGE queue, output
writes the SP HWDGE queue; out is [128, 34304] fp16 partition-major,
all writes contiguous per partition.

Caps (quad 3584 / pair 6656 / single 6656 per core) carry +5-sigma-ish
margins over the occupancy statistics (measured seed-0 input maxima:
3492 / 6547 / 6274). On overflow (adversarial index distribution) fall
back to plan A (replicated-table indirect-DMA gather — slow but correct
for any distribution).

Host packing transposes each chunk's index list so gather slot i =
list[(i%128)*spp + i//128]; host unpacking reshapes chunk tiles back to
list order, casts fp16->f32, and scatters rows to token positions via
the device-row -> sorted-position map (inverse of its own packing).
"""

import numpy as np

import concourse.bacc as bacc
import concourse.mybir as mybir
import concourse.tile as tile
from concourse import bass
from concourse.bass_utils import run_bass_kernel_spmd

NCORES = 8
B, S = 32, 8192
TOKENS = B * S              # 262144
T = TOKENS // NCORES        # 32768 expected tokens per core
V = 100000
D = 128
VSH = V // NCORES           # 12500 vocab rows per core shard

# (replication, slot cap per core, descs per chunk); one chunk of each
# section per queue -> 896+1664+1664 = 4224 descriptors per queue.
SECTIONS = [(4, 3584, 896), (2, 6656, 1664), (1, 6656, 1664)]
NQ = 4
OUTW = sum(r * cap for r, cap, _ in SECTIONS)   # 34304 fp16 cols/partition
_off = 0
SEC_OFF = []
for _r, _cap, _ch in SECTIONS:
    SEC_OFF.append(_off)
    _off += _r * _cap

_compiled = {}


def _build(repeat=1, bufs=6):
    # repeat>1 replicates the body for repeat-slope timing (outputs just
    # get overwritten; timing only).
    nc = bacc.Bacc("TRN2", target_bir_lowering=False, debug=False,
                   num_swdge_queues=NQ)
    idx_d = [
        nc.dram_tensor(f"idx{si}", [NQ, 128, ch // 16], mybir.dt.int16,
                       kind="ExternalInput").ap()
        for si, (_, _, ch) in enumerate(SECTIONS)
    ]
    tab_d = nc.dram_tensor("tab", [VSH, D], mybir.dt.float16,
                           kind="ExternalInput").ap()
    out_d = nc.dram_tensor("out", [128, OUTW], mybir.dt.float16,
                           kind="ExternalOutput").ap()

    with tile.TileContext(nc) as tc:
        with tc.tile_pool(name="idxp", bufs=bufs) as ip, \
             tc.tile_pool(name="pair", bufs=bufs) as pp:
            for _ in range(repeat):
                for si, (rep, cap, ch) in enumerate(SECTIONS):
                    for q in range(NQ):
                        it = ip.tile([128, ch // 16], mybir.dt.int16,
                                     tag=f"i{si}")
                        nc.scalar.dma_start(out=it[:], in_=idx_d[si][q, :, :])
                        pt = pp.tile([128, ch], mybir.dt.float16,
                                     tag=f"p{si}")
                        p3 = pt[:].rearrange("p (s d) -> p s d", d=D)
                        nc.gpsimd.dma_gather(
                            p3, tab_d, it[:],
                            num_idxs=ch, num_idxs_reg=ch, elem_size=D,
                            single_packet=False, queue_num=q)
                        # duplication via repeated DMA writes of the same
                        # gather tile (same total write bytes, zero DVE):
                        # copy r of every slot lands in block r of the
                        # chunk's output range.
                        base = SEC_OFF[si] + q * rep * ch
                        for r in range(rep):
                            nc.sync.dma_start(
                                out=out_d[:, base + r * ch:
                                          base + (r + 1) * ch],
                                in_=pt[:])
    nc.compile()
    return nc


def _build_plan_a():
    G = 8
    NGATH = T // 128
    nc = bacc.Bacc("TRN2", target_bir_lowering=False, debug=False)
    idx_d = nc.dram_tensor("idx", [128, NGATH], mybir.dt.int32,
                           kind="ExternalInput").ap()
    tab_d = nc.dram_tensor("tab", [V, D], mybir.dt.float32,
                           kind="ExternalInput").ap()
    out_d = nc.dram_tensor("out", [T, D], mybir.dt.float32,
                           kind="ExternalOutput").ap()
    with tile.TileContext(nc) as tc:
        with tc.tile_pool(name="data", bufs=3) as dp, \
             tc.tile_pool(name="idxp", bufs=1) as ip:
            it = ip.tile([128, NGATH], mybir.dt.int32)
            nc.sync.dma_start(out=it[:], in_=idx_d[:])
            for c in range(T // (128 * G)):
                dt_ = dp.tile([128, G * D], mybir.dt.float32)
                for g in range(G):
                    nc.gpsimd.indirect_dma_start(
                        out=dt_[:, g * D:(g + 1) * D], out_offset=None,
                        in_=tab_d[:],
                        in_offset=bass.IndirectOffsetOnAxis(
                            ap=it[:, c * G + g:c * G + g + 1], axis=0),
                    )
                dst = out_d[c * G * 128:(c + 1) * G * 128, :] \
                    .rearrange("(g p) d -> p g d", p=128)
                nc.sync.dma_start(
                    out=dst, in_=dt_[:].rearrange("p (g d) -> p g d", g=G))
    nc.compile()
    return nc


def _get_nc(plan):
    if plan not in _compiled:
        _compiled[plan] = _build() if plan == "h" else _build_plan_a()
    return _compiled[plan]


def _wrap16(arr):
    # slot i -> partition i % 16, column i // 16; replicated to 128 partitions
    w = arr.reshape(-1, 16).T
    return np.ascontiguousarray(np.tile(w, (8, 1)))


def _pack_section(rows, cap, ch):
    """rows: [n] sorted row vals -> idx16 [NQ, 128, ch//16] (chunked)."""
    n = rows.shape[0]
    full = np.full(cap, rows[-1] if n else 0, np.int16)
    full[:n] = rows
    idx16 = np.empty((NQ, 128, ch // 16), np.int16)
    spp = ch // 128
    for q in range(NQ):
        slots = full[q * ch:(q + 1) * ch].reshape(128, spp).T.reshape(-1)
        idx16[q] = _wrap16(slots)
    return idx16


def _group_positions(starts_rep, counts, rep):
    """Slot j covers `rep` consecutive sorted positions; returns [n, rep]."""
    n = int(counts.sum())
    within = np.arange(n) - np.repeat(np.cumsum(counts) - counts, counts)
    base = np.repeat(starts_rep, counts) + rep * within
    return base[:, None] + np.arange(rep)[None, :]


def _pack_core(loc):
    """loc: [n] int32 sorted core-local rows -> (idx16s, dev_pos, ns) or
    None on capacity overflow. dev_pos[j] = sorted-order position of real
    device row j (sections in order, each slot's copies consecutive)."""
    vals, cnt = np.unique(loc, return_counts=True)
    starts = np.cumsum(cnt) - cnt
    idx16s, pos_parts, ns = [], [], []
    consumed = np.zeros_like(cnt)
    for si, (rep, cap, ch) in enumerate(SECTIONS):
        k = (cnt - consumed) // rep
        n = int(k.sum())
        if n > cap:
            return None
        rows = np.repeat(vals, k).astype(np.int32)
        pos = _group_positions(starts + consumed, k, rep)
        consumed = consumed + k * rep
        idx16s.append(_pack_section(rows, cap, ch))
        pos_parts.append(pos.reshape(-1))
        ns.append(n)
    dev_pos = np.concatenate(pos_parts)
    return idx16s, dev_pos, ns


def _make_in_maps(X, W, b):
    X = np.asarray(X)
    W = np.asarray(W, dtype=np.float32)
    b = np.asarray(b, dtype=np.float32)
    idx = np.ascontiguousarray(X.reshape(-1).astype(np.int32))
    table32 = np.ascontiguousarray(W.T) + b[None, :]
    table = table32.astype(np.float16)

    order = np.argsort(idx, kind="stable")
    sv = idx[order]
    bounds = np.searchsorted(sv, np.arange(NCORES + 1) * VSH)
    in_maps, metas = [], []
    for c in range(NCORES):
        loc = sv[bounds[c]:bounds[c + 1]] - c * VSH
        packed = _pack_core(loc)
        if packed is None:
            break
        idx16s, dev_pos, ns = packed
        im = {f"idx{si}": idx16s[si] for si in range(len(SECTIONS))}
        im["tab"] = np.ascontiguousarray(table[c * VSH:(c + 1) * VSH])
        in_maps.append(im)
        metas.append((dev_pos, ns))
    else:
        return "h", in_maps, (order, bounds, metas)

    # capacity overflow (pathological index distribution): plan A
    NGATH = T // 128
    in_maps = [
        {"idx": np.ascontiguousarray(
            idx[c * T:(c + 1) * T].reshape(NGATH, 128).T), "tab": table32}
        for c in range(NCORES)
    ]
    return "a", in_maps, None


def _unpack_h(res, meta):
    order, bounds, metas = meta
    out = np.empty((TOKENS, D), np.float32)
    for c in range(NCORES):
        dev_pos, ns = metas[c]
        dev = np.asarray(res.results[c]["out"])          # [128, OUTW] fp16
        parts = []
        for si, (rep, cap, ch) in enumerate(SECTIONS):
            spp = ch // 128
            sec = dev[:, SEC_OFF[si]:SEC_OFF[si] + rep * cap] \
                .reshape(128, NQ, rep, spp, D)
            rows = sec.transpose(1, 0, 3, 2, 4).reshape(rep * cap, D)
            parts.append(rows[:rep * ns[si]])
        rows = np.concatenate(parts)
        sp = bounds[c] + dev_pos
        out[order[sp]] = rows.astype(np.float32)
    return out.reshape(1, TOKENS, D)


def kernel(X, W, b):
    plan, in_maps, meta = _make_in_maps(X, W, b)
    res = run_bass_kernel_spmd(_get_nc(plan), in_maps, list(range(NCORES)))
    if plan == "h":
        return _unpack_h(res, meta)
    out = np.concatenate(
        [res.results[c]["out"] for c in range(NCORES)], axis=0)
    return out.reshape(1, TOKENS, D)
